# revision 44
# baseline (speedup 1.0000x reference)
"""ColBERT late-interaction kernel for 8 TRN2 NeuronCores (Bass/Tile).

Problem (nn_ColBERT): B=64, LQ=32, LP=256, H=768, D=128.
  encode:  x = h @ W + b, normalized over the TOKEN axis (per (batch, d)).
  scores:  sims = einsum('bqd,cpd->bcqp', q, p); masked MaxSim over passage
           tokens, summed over query tokens -> [B, B] per side; output is
           concat([pos_scores, neg_scores], axis=1) -> [64, 128] fp32.

Sharding: passages are sharded across the 8 cores — core j owns pos/neg
passages [8j, 8j+8) and writes a disjoint [64, 16] block of score columns;
the host reassembles [64, 128]. Queries are replicated. No collectives.

Key structure (v2, rewritten from the gather-from-DRAM baseline):
  - Inputs ship PRE-TRANSPOSED from the host ([d-on-partitions, token]
    layout) as fp8e4m3, so no xbar-transpose DMAs and half the bytes;
    chunked straight DMAs overlap the encode.
  - Encode matmuls run in fp8 DoubleRow perf mode (2 contraction rows per
    cell, half the PE cycles). W and b are pre-scaled by 32 on the host to
    keep W out of the fp8 denormal range; the token-axis normalization makes
    the whole pipeline invariant to that scaling. End-to-end rel err ~1.2e-2
    (bf16 variant: ~8e-4) vs the 2e-2 gate; flip FP8_ENCODE off to fall back.
  - One encode only: all 4096 passage + 2048 query tokens are projected once
    (k-outer accumulation, PSUM 2-bank groups); norms via a second ACT
    eviction (func=Square) + per-group DVE segment reduce. The
    compacted passage tokens are then GATHERED IN SBUF from the encoded X
    via gpsimd indirect_copy (quarter-granularity, pipelined with the
    encode) — no re-encode, no second trip to DRAM.
  - Masking via compaction: per-passage unmasked token positions (padded
    with the passage's token 0, which is guaranteed unmasked) are baked into
    a uint16 index tensor on the host.
  - MaxSim: 48 (qt, slot-group) score units of (6,6,4) slots, each <=2 PSUM
    banks so 4 pipeline in flight. Eviction alternates direct DVE reduce_max
    from PSUM with ACT Copy->bf16 + DVE pair-max tree (tensor_tensor runs
    2 elem/cycle on bf16 where reduce_max is fixed at 1). NOTE: hardware
    allows at most one PSUM operand per DVE/ACT instruction, and gpsimd
    supports no generic elementwise ops — both were cross-checked against
    the real walrus compiler, which rejects them (CoreSim does not).
  - Final sum over the 32 queries of each batch is a single PE matmul with a
    block-ones stationary matrix.
"""

import numpy as np
import ml_dtypes

import concourse.bass as bass
from concourse import bacc
import concourse.mybir as mybir
import concourse.tile as tile
from concourse.bass_utils import run_bass_kernel_spmd

BF16 = ml_dtypes.bfloat16

B, LQ, LP, H, D = 64, 32, 256, 768, 128
NCORES = 8
CPC = B // NCORES            # passages per side per core (8)
NP_LOC = 2 * CPC             # local passages (pos+neg) = 16
PTOK = NP_LOC * LP           # 4096 passage tokens per core
QTOK = B * LQ                # 2048 query tokens (replicated)
KCH = H // 128               # 6 contraction chunks
NQT = QTOK // 128            # 16 query tiles

F32 = mybir.dt.float32
BF = mybir.dt.bfloat16
FP8 = mybir.dt.float8e4
U16 = mybir.dt.uint16
AF = mybir.ActivationFunctionType
ALU = mybir.AluOpType
AX = mybir.AxisListType
E4M3 = mybir.dt.np(FP8)

# fp8 DoubleRow encode: h and 32*W ship as fp8e4m3 (halves input DMA bytes
# and PE encode cycles). The projection is scale-invariant under x -> 32x
# because of the token-axis normalization, so scaling W/b by 32 just moves
# the weights out of the fp8 denormal range. Measured end-to-end rel err
# ~1.1e-2 (vs 8e-4 for bf16), within the 2e-2 gate.
FP8_ENCODE = True

# Eviction mode per score unit, cycled. Each unit's [128, ns*n_pad] f32
# PSUM score block must be max-reduced per slot; reduce_max runs at
# 1 elem/cycle on DVE only, and DVE/ACT can read at most ONE PSUM operand
# per instruction (HW port limit), so the work is spread over 3 pipelines:
#   D: direct DVE reduce_max from PSUM                  (DVE ~1.0us)
#   A: ACT Copy->bf16, DVE pair-max L1+L2, DVE reduce   (ACT 0.75, DVE 0.8)
#   G: ACT Copy->bf16, gpsimd pair-max L1+L2, DVE reduce (ACT 0.75,
#      Pool 1.0, DVE 0.3)
EVICT_PATTERN = "AAAADAAAAAAADAAA"
GP_TREES = False      # pair-max trees on gpsimd (G mode); else they become A
GP_Q2SCALE = False    # Q2 scale-apply on gpsimd; else DVE


def build_program(n_pad: int, reps: int = 1) -> bass.Bass:
    ni = NP_LOC * n_pad
    hdt = FP8 if FP8_ENCODE else BF
    nc = bacc.Bacc(None)
    hqT = nc.declare_dram_parameter("hqT", [128, KCH, QTOK], hdt, isOutput=False)
    hpT = nc.declare_dram_parameter("hpT", [128, KCH, PTOK], hdt, isOutput=False)
    Wd = nc.declare_dram_parameter("W", [128, KCH, 128], hdt, isOutput=False)
    bd = nc.declare_dram_parameter("b", [D, 1], F32, isOutput=False)
    cidx = nc.declare_dram_parameter("cidx", [128, ni // 16], U16, isOutput=False)
    outd = nc.declare_dram_parameter("out", [B, NP_LOC], F32, isOutput=True)

    with tile.TileContext(nc) as tc:
        for _ in range(reps):
            _emit_body(nc, tc, hqT, hpT, Wd, bd, cidx, outd, n_pad)
    nc.finalize()
    return nc


def _emit_body(nc, tc, hqT, hpT, Wd, bd, cidx, outd, n_pad):
    ni = NP_LOC * n_pad
    nhalf = ni // 2
    with (
        tc.tile_pool(name="const", bufs=1) as constp,
        tc.tile_pool(name="big", bufs=1) as bigp,
    ):
        # ---- constants -------------------------------------------------
        Wt = constp.tile([128, KCH, 128], FP8 if FP8_ENCODE else BF)
        nc.sync.dma_start(Wt[:], Wd[:])
        bcol = constp.tile([128, 1], F32)
        nc.sync.dma_start(bcol[:], bd[:])
        A = constp.tile([128, 4], F32)          # block-ones for query sums
        nc.vector.memset(A[:], 0.0)
        for i in range(4):
            nc.vector.memset(A[32 * i:32 * (i + 1), i:i + 1], 1.0)

        # encoded tokens, [d, token]
        Xq = bigp.tile([128, QTOK], BF)
        Xp = bigp.tile([128, PTOK], BF)
        # norms / scales
        nsqQ = bigp.tile([128, B], F32)
        nsqP = bigp.tile([128, NP_LOC], F32)
        sclQ = bigp.tile([128, B], F32)
        sclP = bigp.tile([128, NP_LOC], F32)
        # scaled operands for the score matmuls
        Q2 = bigp.tile([128, QTOK], BF)
        P2r = bigp.tile([128, ni], BF)          # gathered, unscaled
        P2 = bigp.tile([128, ni], BF)           # gathered, scaled
        Mx = bigp.tile([128, NQT * NP_LOC], F32)

        # ---- input DMAs (chunked so encode can start early) ------------
        hdt = FP8 if FP8_ENCODE else BF
        hq_sb = bigp.tile([128, KCH, QTOK], hdt)
        hp_sb = bigp.tile([128, KCH, PTOK], hdt)
        for t0 in range(0, QTOK, 1024):
            nc.sync.dma_start(hq_sb[:, :, t0:t0 + 1024], hqT[:, :, t0:t0 + 1024])
        idxt = constp.tile([128, ni // 16], U16)
        nc.sync.dma_start(idxt[:], cidx[:])     # not needed until the gather
        for t0 in range(0, PTOK, 1024):
            nc.sync.dma_start(hp_sb[:, :, t0:t0 + 1024], hpT[:, :, t0:t0 + 1024])

        def scale_from_nsq(scl, nsq, n):
            # scl = 1 / max(sqrt(nsq), 1e-12)
            nc.scalar.sqrt(scl[:, :n], nsq[:, :n])
            nc.vector.tensor_scalar_max(scl[:, :n], scl[:, :n], 1e-12)
            nc.vector.reciprocal(scl[:, :n], scl[:, :n])

        # ---- encode: 6 groups of 1024 tokens (2 q + 4 p) ---------------
        with (
            tc.tile_pool(name="xps", bufs=3, space="PSUM") as xpsp,
            tc.tile_pool(name="sq", bufs=3) as sqp,
        ):
            for g in range(6):
                if g < 2:
                    src, X, off = hq_sb, Xq, g * 1024
                else:
                    src, X, off = hp_sb, Xp, (g - 2) * 1024
                ps = xpsp.tile([128, 1024], F32, tag="x")
                if FP8_ENCODE:
                    for c in range(KCH // 2):
                        for n in range(2):
                            nc.tensor.matmul(
                                ps[:, n * 512:(n + 1) * 512],
                                Wt[:, 2 * c:2 * c + 2, :],
                                src[:, 2 * c:2 * c + 2,
                                    off + n * 512: off + (n + 1) * 512],
                                start=(c == 0), stop=(c == KCH // 2 - 1),
                                perf_mode=mybir.MatmulPerfMode.DoubleRow,
                            )
                else:
                    for k in range(KCH):
                        for n in range(2):
                            nc.tensor.matmul(
                                ps[:, n * 512:(n + 1) * 512],
                                Wt[:, k, :],
                                src[:, k, off + n * 512: off + (n + 1) * 512],
                                start=(k == 0), stop=(k == KCH - 1),
                            )
                nc.scalar.activation(
                    X[:, off:off + 1024], ps[:], AF.Identity,
                    bias=bcol[:, 0:1], scale=1.0,
                )
                # squares on ACT (second PSUM eviction with func=Square) —
                # DVE is the scarcest engine overall; norms only need the
                # reduce there
                sq = sqp.tile([128, 1024], BF, tag="sq")
                nc.scalar.activation(
                    sq[:], ps[:], AF.Square, bias=bcol[:, 0:1], scale=1.0)
                if g < 2:
                    nc.vector.reduce_sum(
                        nsqQ[:, g * 32:(g + 1) * 32],
                        sq[:].rearrange("p (b l) -> p b l", b=32), axis=AX.X)
                else:
                    nc.vector.reduce_sum(
                        nsqP[:, (g - 2) * 4:(g - 1) * 4],
                        sq[:].rearrange("p (c l) -> p c l", c=4), axis=AX.X)

                if g == 1:
                    # queries done: scales + scaled Q2 (on gpsimd)
                    scale_from_nsq(sclQ, nsqQ, B)
                    if GP_Q2SCALE:
                        nc.gpsimd.scalar_tensor_tensor(
                            Q2[:].rearrange("p (b l) -> p b l", b=B),
                            Xq[:].rearrange("p (b l) -> p b l", b=B),
                            1.0,
                            sclQ[:].unsqueeze(2).broadcast_to([128, B, LQ]),
                            ALU.mult, ALU.mult,
                        )
                    else:
                        nc.vector.tensor_tensor(
                            Q2[:].rearrange("p (b l) -> p b l", b=B),
                            Xq[:].rearrange("p (b l) -> p b l", b=B),
                            sclQ[:].unsqueeze(2).broadcast_to([128, B, LQ]),
                            ALU.mult,
                        )
                if g >= 2:
                    # a quarter of the passages done: scale + gather + apply
                    q4 = g - 2
                    c0, c1 = q4 * 4, q4 * 4 + 4
                    nq = 4 * n_pad
                    scale_from_nsq(sclP[:, c0:c1], nsqP[:, c0:c1], 4)
                    nc.gpsimd.indirect_copy(
                        P2r[:, q4 * nq:(q4 + 1) * nq],
                        Xp[:, q4 * 1024:(q4 + 1) * 1024],
                        idxt[:, q4 * (nq // 16):(q4 + 1) * (nq // 16)],
                        i_know_ap_gather_is_preferred=True,
                    )
                    nc.vector.tensor_tensor(
                        P2[:, q4 * nq:(q4 + 1) * nq].rearrange(
                            "p (c n) -> p c n", c=4),
                        P2r[:, q4 * nq:(q4 + 1) * nq].rearrange(
                            "p (c n) -> p c n", c=4),
                        sclP[:, c0:c1].unsqueeze(2).broadcast_to([128, 4, n_pad]),
                        ALU.mult,
                    )

        # ---- scores: 32 (qt, half) units -------------------------------
        with (
            tc.tile_pool(name="sps", bufs=4, space="PSUM") as spsp,
            tc.tile_pool(name="sc", bufs=6) as scp,
            tc.tile_pool(name="tr", bufs=6) as trp,
        ):
            n2, n4 = n_pad // 2, n_pad // 4
            # 48 units of (6, 6, 4) slots: each PSUM tile is <= 2 banks so
            # four are in flight. Third-major order: the first 16 units only
            # need the first two gather quarters.
            units = [(qt, s0, ns)
                     for (s0, ns) in ((0, 6), (6, 6), (12, 4))
                     for qt in range(NQT)]
            for u, (qt, s0, ns) in enumerate(units):
                nu = ns * n_pad
                ps = spsp.tile([128, 6 * n_pad], F32, tag="s")
                for c0 in range(0, nu, 512):
                    c1 = min(c0 + 512, nu)
                    nc.tensor.matmul(
                        ps[:, c0:c1],
                        Q2[:, qt * 128:(qt + 1) * 128],
                        P2[:, s0 * n_pad + c0: s0 * n_pad + c1],
                        start=True, stop=True,
                    )
                mx_out = Mx[:, qt * NP_LOC + s0: qt * NP_LOC + s0 + ns]
                mode = EVICT_PATTERN[u % len(EVICT_PATTERN)]
                if mode == "G" and not GP_TREES:
                    mode = "A"
                ps3 = ps[:, :nu].rearrange("p (s n) -> p s n", n=n_pad)
                if mode == "D":
                    nc.vector.reduce_max(mx_out, ps3, axis=AX.X)
                    continue
                t1 = trp.tile([128, ns, n2], BF, tag="t1")
                t2 = trp.tile([128, ns, n4], BF, tag="t2")
                sc = scp.tile([128, 6 * n_pad], BF, tag="sc")
                nc.scalar.activation(sc[:, :nu], ps[:, :nu], AF.Copy,
                                     scale=1.0)
                sc3 = sc[:, :nu].rearrange("p (s n) -> p s n", n=n_pad)
                if mode == "A":
                    nc.vector.tensor_tensor(
                        t1[:], sc3[:, :, :n2], sc3[:, :, n2:], ALU.max)
                    nc.vector.tensor_tensor(
                        t2[:], t1[:, :, :n4], t1[:, :, n4:], ALU.max)
                else:  # G: pair-max on gpsimd, final reduce on DVE
                    nc.gpsimd.scalar_tensor_tensor(
                        t1[:], sc3[:, :, :n2], 1.0, sc3[:, :, n2:],
                        ALU.mult, ALU.max)
                    nc.gpsimd.scalar_tensor_tensor(
                        t2[:], t1[:, :, :n4], 1.0, t1[:, :, n4:],
                        ALU.mult, ALU.max)
                nc.vector.reduce_max(mx_out, t2[:], axis=AX.X)

        # ---- sum over the 32 queries of each batch via PE --------------
        with tc.tile_pool(name="fin", bufs=1, space="PSUM") as finp:
            fin = finp.tile([4, NQT * NP_LOC], F32)
            nc.tensor.matmul(fin[:], A[:], Mx[:], start=True, stop=True)
            fsb = bigp.tile([4, NQT * NP_LOC], F32)
            nc.vector.tensor_copy(fsb[:], fin[:])
        nc.sync.dma_start(
            outd.rearrange("(t b4) c -> b4 t c", b4=4),
            fsb[:].rearrange("p (t c) -> p t c", t=NQT),
        )


def _prep_inputs(query_hidden, positive_hidden, negative_hidden, W, b,
                 positive_filter_mask, negative_filter_mask):
    """Returns (in_maps, n_pad)."""
    hdt = E4M3 if FP8_ENCODE else BF16
    wscale = 32.0 if FP8_ENCODE else 1.0
    hq = np.asarray(query_hidden, np.float32).reshape(QTOK, H)
    # [QTOK, 6, 128] -> [128, 6, QTOK]
    hqT = np.ascontiguousarray(
        hq.reshape(QTOK, KCH, 128).transpose(2, 1, 0)).astype(hdt)
    Wt = np.ascontiguousarray(
        np.asarray(W, np.float32).reshape(KCH, 128, D).transpose(1, 0, 2)
        * wscale
    ).astype(hdt)
    bcol = np.ascontiguousarray(
        np.asarray(b, np.float32).reshape(D, 1) * wscale)
    pos = np.asarray(positive_hidden, np.float32)
    neg = np.asarray(negative_hidden, np.float32)
    pmask = np.asarray(positive_filter_mask).astype(bool)
    nmask = np.asarray(negative_filter_mask).astype(bool)

    core_masks = []
    n_max = 1
    for j in range(NCORES):
        sl = slice(j * CPC, (j + 1) * CPC)
        m = np.concatenate([pmask[sl], nmask[sl]], axis=0)   # [16, 256]
        core_masks.append(m)
        n_max = max(n_max, int(m.sum(axis=1).max()))
    n_pad = -(-n_max // 32) * 32
    ni = NP_LOC * n_pad

    in_maps = []
    for j in range(NCORES):
        sl = slice(j * CPC, (j + 1) * CPC)
        hp = np.concatenate(
            [pos[sl].reshape(CPC * LP, H), neg[sl].reshape(CPC * LP, H)], axis=0)
        hpT = np.ascontiguousarray(
            hp.reshape(PTOK, KCH, 128).transpose(2, 1, 0)).astype(hdt)
        m = core_masks[j]
        idx = np.zeros((NP_LOC, n_pad), np.uint16)
        for c in range(NP_LOC):
            pos_c = np.nonzero(m[c])[0]
            # token index within c's quarter-window of Xp (4 passages each)
            base = c * LP - (c // 4) * 1024
            idx[c, :len(pos_c)] = pos_c + base
            idx[c, len(pos_c):] = base                        # dup token 0
        flat = idx.reshape(ni)
        wrapped = np.tile(flat.reshape(ni // 16, 16).T, (8, 1)).copy()
        in_maps.append({
            "hqT": hqT, "hpT": hpT, "W": Wt, "b": bcol, "cidx": wrapped,
        })
    return in_maps, n_pad


def _assemble(results):
    out = np.zeros((B, 2 * B), np.float32)
    for j in range(NCORES):
        o = np.asarray(results[j]["out"], np.float32)      # [64, 16]
        out[:, j * CPC:(j + 1) * CPC] = o[:, :CPC]
        out[:, B + j * CPC:B + (j + 1) * CPC] = o[:, CPC:]
    return out


def kernel(query_hidden, positive_hidden, negative_hidden, W, b,
           positive_filter_mask, negative_filter_mask):
    in_maps, n_pad = _prep_inputs(query_hidden, positive_hidden, negative_hidden,
                                  W, b, positive_filter_mask, negative_filter_mask)
    nc = build_program(n_pad)
    res = run_bass_kernel_spmd(nc, in_maps, list(range(NCORES)))
    return _assemble(res.results)
